# revision 5
# baseline (speedup 1.0000x reference)
"""Trainium2 Bass kernel for nn_BiAttention (sparse_attention).

Math: the reference's attention matrix is rank-1 plus a mask bias:
    att[b,l,m] = input_dot[b,l] + s[b,m],  s[m] = memory[m]@w_mem1 - 1e30*(1-mask[m])
Row softmax over m is invariant to the per-row constant input_dot[b,l], so
    weight_one[b,l,:] = softmax_m(s)            (same for every l)
    output_one[b,l,:] = v_b := softmax_m(s) @ (memory @ W_mem2.T + b_mem2)
Likewise max_m att[b,l,m] = input_dot[b,l] + const, so
    weight_two[b,0,:] = softmax_l(input_dot)
    output_two[b,0,:] = softmax_l(input_dot) @ inp2
The output [N, 4*Ld, d] row blocks are:
    [0:2048]    inp2 = input @ W_in2.T + b_in2
    [2048:4096] v_b broadcast
    [4096:6144] inp2 * v_b
    [6144:8192] (output_two * v_b) broadcast

Sharding: pure data parallel, one batch element per NeuronCore (8 cores).

The only heavy compute is inp2 (one [2048,1024]@[1024,1024] matmul per
core) done in float32r on the PE (input tiles transposed on-chip via PE
transpose), plus 32 MB/core of output DMA, which is the roofline term.
"""

import numpy as np

import concourse.bass as bass
import concourse.tile as tile
from concourse import bacc, mybir
from concourse.bass_utils import run_bass_kernel_spmd
from concourse.masks import make_identity

F32 = mybir.dt.float32
F32R = mybir.dt.float32r
AX = mybir.AxisListType
OP = mybir.AluOpType
EXP = mybir.ActivationFunctionType.Exp

P = 128
BSZ, LD, LM, HID = 8, 2048, 512, 1024
KT = HID // P          # 8 hidden-dim chunks
LT = LD // P           # 16 l tiles
MT = LM // P           # 4 memory tiles
N_CORES = 8

_NC_CACHE = None


def _rank1_bcast(nc, pool, psum_pool, ones_row, row_ap, name):
    """Broadcast a [1, HID] SBUF row across 128 partitions -> [128, HID].

    Uses K=1 fp32 matmuls (lhsT = ones [1,128]) which reproduce the row
    exactly. Returns the [128, HID] SBUF tile.
    """
    bc = pool.tile([P, HID], F32, tag=name)
    for h in range(2):
        ps = psum_pool.tile([P, 512], F32, tag="out")
        nc.tensor.matmul(
            ps[:], ones_row[:], row_ap[:, h * 512:(h + 1) * 512],
            start=True, stop=True,
        )
        nc.scalar.copy(bc[:, h * 512:(h + 1) * 512], ps[:])
    return bc


def _build_nc():
    nc = bacc.Bacc("TRN2", target_bir_lowering=False, num_devices=N_CORES)

    inp_d = nc.dram_tensor("input", [LD, HID], F32, kind="ExternalInput").ap()
    mem_d = nc.dram_tensor("memory", [LM, HID], F32, kind="ExternalInput").ap()
    mask_d = nc.dram_tensor("mask", [1, LM], F32, kind="ExternalInput").ap()
    wi1_d = nc.dram_tensor("w_in1", [1, HID], F32, kind="ExternalInput").ap()
    wm1_d = nc.dram_tensor("w_mem1", [1, HID], F32, kind="ExternalInput").ap()
    Wi2_d = nc.dram_tensor("W_in2", [HID, HID], F32, kind="ExternalInput").ap()
    bi2_d = nc.dram_tensor("b_in2", [1, HID], F32, kind="ExternalInput").ap()
    Wm2_d = nc.dram_tensor("W_mem2", [HID, HID], F32, kind="ExternalInput").ap()
    bm2_d = nc.dram_tensor("b_mem2", [1, HID], F32, kind="ExternalInput").ap()
    out_d = nc.dram_tensor("out", [4 * LD, HID], F32, kind="ExternalOutput").ap()

    with tile.TileContext(nc) as tc:
        with (
            tc.tile_pool(name="const", bufs=1) as cpool,
            tc.tile_pool(name="bc", bufs=1) as bcpool,
            tc.tile_pool(name="wT", bufs=1) as wtpool,
            tc.tile_pool(name="wnat", bufs=1) as wnatpool,
            tc.tile_pool(name="mem", bufs=1) as mempool,
            tc.tile_pool(name="rows", bufs=1) as rowpool,
            tc.tile_pool(name="at", bufs=3) as atpool,
            tc.tile_pool(name="intp", bufs=3) as intpool,
            tc.tile_pool(name="inp2", bufs=3) as inp2pool,
            tc.tile_pool(name="prod", bufs=2) as prodpool,
            tc.tile_pool(name="ttr", bufs=2) as ttrpool,
            tc.tile_pool(name="small", bufs=4) as smallpool,
            tc.tile_pool(name="ptr", bufs=2, space="PSUM") as ptrpool,
            tc.tile_pool(name="pout", bufs=4, space="PSUM") as poutpool,
            tc.tile_pool(name="psS", bufs=1, space="PSUM") as pspool,
        ):
            # ---------------- constants / small loads ----------------
            ident = cpool.tile([P, P], F32)
            make_identity(nc, ident)
            ones_row = cpool.tile([1, P], F32)
            nc.vector.memset(ones_row[:], 1.0)
            ones_col = cpool.tile([P, 1], F32)
            nc.vector.memset(ones_col[:], 1.0)

            wi1_row = rowpool.tile([1, HID], F32, tag="wi1r")
            nc.sync.dma_start(wi1_row[:], wi1_d[:])
            wm1_row = rowpool.tile([1, HID], F32, tag="wm1r")
            nc.sync.dma_start(wm1_row[:], wm1_d[:])
            bi2_row = rowpool.tile([1, HID], F32, tag="bi2r")
            nc.sync.dma_start(bi2_row[:], bi2_d[:])
            bm2_row = rowpool.tile([1, HID], F32, tag="bm2r")
            nc.sync.dma_start(bm2_row[:], bm2_d[:])
            mask_col = cpool.tile([P, MT], F32)
            nc.sync.dma_start(mask_col[:], mask_d.rearrange("1 (o p) -> p o", p=P))

            # broadcasts of rows used inside per-partition ops (DMA bcast)
            wi1_bc = bcpool.tile([P, HID], F32, tag="wi1bc")
            nc.sync.dma_start(wi1_bc[:], wi1_d.to_broadcast([P, HID]))
            wm1_bc = bcpool.tile([P, HID], F32, tag="wm1bc")
            nc.sync.dma_start(wm1_bc[:], wm1_d.to_broadcast([P, HID]))
            bi2_bc = bcpool.tile([P, HID], F32, tag="bi2bc")
            nc.sync.dma_start(bi2_bc[:], bi2_d.to_broadcast([P, HID]))

            # ---------------- W_in2^T via PE transpose ----------------
            wnat = wnatpool.tile([P, KT, HID], F32, tag="wnat")
            nc.sync.dma_start(wnat[:], Wi2_d.rearrange("(o p) d -> p o d", p=P))
            w2t = wtpool.tile([P, KT, HID], F32R)
            for k in range(KT):
                for ojh in range(2):
                    ps = ptrpool.tile([P, 512], F32, tag="tr")
                    for j in range(4):
                        oj = ojh * 4 + j
                        nc.tensor.transpose(
                            ps[:, j * P:(j + 1) * P],
                            wnat[:, oj, k * P:(k + 1) * P],
                            ident,
                        )
                    nc.scalar.copy(w2t[:, k, ojh * 512:(ojh + 1) * 512], ps[:])

            # ---------------- v path ----------------
            mem_t = mempool.tile([P, MT, HID], F32, tag="memt")
            nc.sync.dma_start(mem_t[:], mem_d.rearrange("(j p) d -> p j d", p=P))

            # s[m] = memory[m]@w_mem1 - 1e30*(1-mask[m]) ; column layout [128, MT]
            s_col = smallpool.tile([P, MT], F32, tag="scol")
            for j in range(MT):
                scr = ttrpool.tile([P, HID], F32, tag="ttr")
                nc.vector.tensor_mul(scr[:], mem_t[:, j, :], wm1_bc[:])
                nc.vector.tensor_reduce(s_col[:, j:j + 1], scr[:], AX.X, OP.add)
            msk = smallpool.tile([P, MT], F32, tag="msk")
            nc.vector.tensor_scalar(msk[:], mask_col[:], -1.0, 1e30, OP.add, OP.mult)
            nc.vector.tensor_add(s_col[:], s_col[:], msk[:])
            # e_s = exp(s) (no max-shift needed: |s| <= ~5 for live entries)
            e_s = smallpool.tile([P, MT], F32, tag="es")
            nc.scalar.activation(e_s[:], s_col[:], EXP)

            # P_un[o] = sum_m e_s[m] * memory[m, o]  (PE, fp32)
            pun_ps = [ptrpool.tile([1, 512], F32, tag="tr", name=f"pun{h}") for h in range(2)]
            for j in range(MT):
                for h in range(2):
                    nc.tensor.matmul(
                        pun_ps[h][:], e_s[:, j:j + 1],
                        mem_t[:, j, h * 512:(h + 1) * 512],
                        start=(j == 0), stop=(j == MT - 1),
                        skip_group_check=True,
                    )
            pun_row = rowpool.tile([1, HID], F32, tag="punr")
            for h in range(2):
                nc.scalar.copy(pun_row[:, h * 512:(h + 1) * 512], pun_ps[h][:])

            # Z_s = sum_m e_s[m]
            zs_ps = poutpool.tile([1, MT], F32, tag="out")
            nc.tensor.matmul(zs_ps[:], ones_col[:], e_s[:], start=True, stop=True)
            zs_row = smallpool.tile([1, 1], F32, tag="zs")
            nc.vector.tensor_reduce(zs_row[:], zs_ps[:], AX.X, OP.add)
            rzs = smallpool.tile([1, 1], F32, tag="rzs")
            nc.vector.reciprocal(rzs[:], zs_row[:])

            # vT[o] = sum_d W_mem2[o, d] * P_un[d]  (DVE dot vs natural W_mem2)
            pun_bc = _rank1_bcast(nc, bcpool, poutpool, ones_row, pun_row, "punbc")
            wm2nat = wnatpool.tile([P, KT, HID], F32, tag="wnat")
            nc.sync.dma_start(wm2nat[:], Wm2_d.rearrange("(o p) d -> p o d", p=P))
            vt_col = smallpool.tile([P, KT], F32, tag="vt")
            for j2 in range(KT):
                scr = ttrpool.tile([P, HID], F32, tag="ttr")
                nc.vector.tensor_mul(scr[:], wm2nat[:, j2, :], pun_bc[:])
                nc.vector.tensor_reduce(vt_col[:, j2:j2 + 1], scr[:], AX.X, OP.add)
            # v = vT/Z_s + b_mem2 as a [1, HID] row
            v_row_u = rowpool.tile([1, HID], F32, tag="vrowu")
            for j2 in range(KT):
                nc.sync.dma_start(
                    v_row_u[:, j2 * P:(j2 + 1) * P], vt_col[:, j2:j2 + 1]
                )
            v_row = rowpool.tile([1, HID], F32, tag="vrow")
            nc.vector.tensor_scalar(v_row[:], v_row_u[:], rzs[:], None, OP.mult)
            nc.vector.tensor_add(v_row[:], v_row[:], bm2_row[:])
            v_bc = _rank1_bcast(nc, bcpool, poutpool, ones_row, v_row, "vbc")

            # ---------------- main loop ----------------
            e_tile = cpool.tile([P, LT], F32R)
            e_f32 = cpool.tile([P, LT], F32)
            s_ps = [pspool.tile([1, 512], F32, tag=f"s{h}", name=f"s{h}") for h in range(2)]

            for i in range(LT):
                at = atpool.tile([P, HID], F32, tag="at")
                nc.sync.dma_start(at[:], inp_d[i * P:(i + 1) * P, :])

                # input tile transpose -> [128(d), k, 128(l)] rounded to f32r
                int_t = intpool.tile([P, KT, P], F32R, tag="int")
                for kh in range(2):
                    ps = ptrpool.tile([P, 512], F32, tag="tr")
                    for j in range(4):
                        k = kh * 4 + j
                        nc.tensor.transpose(
                            ps[:, j * P:(j + 1) * P],
                            at[:, k * P:(k + 1) * P],
                            ident,
                        )
                    nc.scalar.copy(
                        int_t[:, kh * 4:(kh + 1) * 4, :].rearrange("p a b -> p (a b)"),
                        ps[:],
                    )

                # idot -> e (exp without shift; |idot| <= ~4)
                scr = ttrpool.tile([P, HID], F32, tag="ttr")
                idc = smallpool.tile([P, 1], F32, tag="idc")
                nc.vector.tensor_mul(scr[:], at[:], wi1_bc[:])
                nc.vector.tensor_reduce(idc[:], scr[:], AX.X, OP.add)
                nc.scalar.activation(e_f32[:, i:i + 1], idc[:], EXP)
                nc.scalar.copy(e_tile[:, i:i + 1], e_f32[:, i:i + 1])

                # inp2 tile: f32r matmul + bias
                inp2_sb = inp2pool.tile([P, HID], F32R, tag="inp2")
                for h in range(2):
                    pso = poutpool.tile([P, 512], F32, tag="out")
                    for k in range(KT):
                        nc.tensor.matmul(
                            pso[:], int_t[:, k, :],
                            w2t[:, k, h * 512:(h + 1) * 512],
                            start=(k == 0), stop=(k == KT - 1),
                        )
                    nc.vector.tensor_add(
                        inp2_sb[:, h * 512:(h + 1) * 512], pso[:],
                        bi2_bc[:, h * 512:(h + 1) * 512],
                    )
                    # S += e_i^T inp2_i  (accumulates over the whole loop)
                    nc.tensor.matmul(
                        s_ps[h][:], e_tile[:, i:i + 1],
                        inp2_sb[:, h * 512:(h + 1) * 512],
                        start=(i == 0), stop=(i == LT - 1),
                        skip_group_check=True,
                    )

                prod_sb = prodpool.tile([P, HID], F32, tag="prod")
                nc.vector.tensor_mul(prod_sb[:], inp2_sb.bitcast(F32)[:], v_bc[:])

                nc.sync.dma_start(
                    out_d[i * P:(i + 1) * P, :], inp2_sb.bitcast(F32)[:]
                )
                nc.sync.dma_start(
                    out_d[2 * LD + i * P:2 * LD + (i + 1) * P, :], prod_sb[:]
                )
                nc.sync.dma_start(out_d[LD + i * P:LD + (i + 1) * P, :], v_bc[:])

            # ---------------- tail: out2 and u rows ----------------
            z_ps = poutpool.tile([1, LT], F32, tag="out")
            nc.tensor.matmul(z_ps[:], ones_col[:], e_f32[:], start=True, stop=True)
            z_row = smallpool.tile([1, LT], F32, tag="zrow")
            nc.scalar.copy(z_row[:], z_ps[:])
            z_sb = smallpool.tile([1, 1], F32, tag="z")
            nc.vector.tensor_reduce(z_sb[:], z_row[:], AX.X, OP.add)
            rz = smallpool.tile([1, 1], F32, tag="rz")
            nc.vector.reciprocal(rz[:], z_sb[:])
            s_row = rowpool.tile([1, HID], F32, tag="srow")
            for h in range(2):
                nc.scalar.copy(s_row[:, h * 512:(h + 1) * 512], s_ps[h][:])
            u_row = rowpool.tile([1, HID], F32, tag="urow")
            nc.vector.tensor_scalar(u_row[:], s_row[:], rz[:], None, OP.mult)
            nc.vector.tensor_mul(u_row[:], u_row[:], v_row[:])
            u_bc = _rank1_bcast(nc, bcpool, poutpool, ones_row, u_row, "ubc")
            for i in range(LT):
                nc.sync.dma_start(
                    out_d[3 * LD + i * P:3 * LD + (i + 1) * P, :], u_bc[:]
                )

    nc.finalize()
    return nc


def _get_nc():
    global _NC_CACHE
    if _NC_CACHE is None:
        _NC_CACHE = _build_nc()
    return _NC_CACHE


def kernel(**inputs) -> np.ndarray:
    nc = _get_nc()
    inp = np.asarray(inputs["input"], dtype=np.float32)
    mem = np.asarray(inputs["memory"], dtype=np.float32)
    mask = np.asarray(inputs["mask"], dtype=np.float32)
    w_in1 = np.ascontiguousarray(np.asarray(inputs["w_in1"], np.float32).reshape(1, HID))
    w_mem1 = np.ascontiguousarray(np.asarray(inputs["w_mem1"], np.float32).reshape(1, HID))
    W_in2 = np.ascontiguousarray(np.asarray(inputs["W_in2"], np.float32))
    b_in2 = np.ascontiguousarray(np.asarray(inputs["b_in2"], np.float32).reshape(1, HID))
    W_mem2 = np.ascontiguousarray(np.asarray(inputs["W_mem2"], np.float32))
    b_mem2 = np.ascontiguousarray(np.asarray(inputs["b_mem2"], np.float32).reshape(1, HID))

    in_maps = []
    for b in range(N_CORES):
        in_maps.append({
            "input": np.ascontiguousarray(inp[b]),
            "memory": np.ascontiguousarray(mem[b]),
            "mask": np.ascontiguousarray(mask[b].reshape(1, LM)),
            "w_in1": w_in1,
            "w_mem1": w_mem1,
            "W_in2": W_in2,
            "b_in2": b_in2,
            "W_mem2": W_mem2,
            "b_mem2": b_mem2,
        })

    res = run_bass_kernel_spmd(nc, in_maps, core_ids=list(range(N_CORES)))
    return np.stack([res.results[c]["out"] for c in range(N_CORES)], axis=0)


# revision 7
# speedup vs baseline: 1.0409x; 1.0409x over previous
"""Trainium2 Bass kernel for nn_BiAttention (sparse_attention).

Math: the reference's attention matrix is rank-1 plus a mask bias:
    att[b,l,m] = input_dot[b,l] + s[b,m],  s[m] = memory[m]@w_mem1 - 1e30*(1-mask[m])
Row softmax over m is invariant to the per-row constant input_dot[b,l], so
    weight_one[b,l,:] = softmax_m(s)            (same for every l)
    output_one[b,l,:] = v_b := softmax_m(s) @ (memory @ W_mem2.T + b_mem2)
Likewise max_m att[b,l,m] = input_dot[b,l] + const, so
    weight_two[b,0,:] = softmax_l(input_dot)
    output_two[b,0,:] = softmax_l(input_dot) @ inp2
The output [N, 4*Ld, d] row blocks are:
    [0:2048]    inp2 = input @ W_in2.T + b_in2
    [2048:4096] v_b broadcast
    [4096:6144] inp2 * v_b
    [6144:8192] (output_two * v_b) broadcast

Sharding: pure data parallel, one batch element per NeuronCore (8 cores).

The only heavy compute is inp2 (one [2048,1024]@[1024,1024] matmul per
core) done in float32r on the PE (input tiles transposed on-chip via PE
transpose), plus 32 MB/core of output DMA, which is the roofline term.
"""

import numpy as np

import concourse.bass as bass
import concourse.tile as tile
from concourse import bacc, mybir
from concourse.bass_utils import run_bass_kernel_spmd
from concourse.masks import make_identity

F32 = mybir.dt.float32
F32R = mybir.dt.float32r
AX = mybir.AxisListType
OP = mybir.AluOpType
EXP = mybir.ActivationFunctionType.Exp

P = 128
BSZ, LD, LM, HID = 8, 2048, 512, 1024
KT = HID // P          # 8 hidden-dim chunks
LT = LD // P           # 16 l tiles
MT = LM // P           # 4 memory tiles
N_CORES = 8

_NC_CACHE = None


def _rank1_bcast(nc, pool, psum_pool, ones_row, row_ap, name):
    """Broadcast a [1, HID] SBUF row across 128 partitions -> [128, HID].

    Uses K=1 fp32 matmuls (lhsT = ones [1,128]) which reproduce the row
    exactly. Returns the [128, HID] SBUF tile.
    """
    bc = pool.tile([P, HID], F32, tag=name)
    for h in range(2):
        ps = psum_pool.tile([P, 512], F32, tag="out")
        nc.tensor.matmul(
            ps[:], ones_row[:], row_ap[:, h * 512:(h + 1) * 512],
            start=True, stop=True,
        )
        nc.scalar.copy(bc[:, h * 512:(h + 1) * 512], ps[:])
    return bc


def _build_nc():
    nc = bacc.Bacc("TRN2", target_bir_lowering=False, num_devices=N_CORES)

    inp_d = nc.dram_tensor("input", [LD, HID], F32, kind="ExternalInput").ap()
    mem_d = nc.dram_tensor("memory", [LM, HID], F32, kind="ExternalInput").ap()
    mask_d = nc.dram_tensor("mask", [1, LM], F32, kind="ExternalInput").ap()
    wi1_d = nc.dram_tensor("w_in1", [1, HID], F32, kind="ExternalInput").ap()
    wm1_d = nc.dram_tensor("w_mem1", [1, HID], F32, kind="ExternalInput").ap()
    Wi2_d = nc.dram_tensor("W_in2", [HID, HID], F32, kind="ExternalInput").ap()
    bi2_d = nc.dram_tensor("b_in2", [1, HID], F32, kind="ExternalInput").ap()
    Wm2_d = nc.dram_tensor("W_mem2", [HID, HID], F32, kind="ExternalInput").ap()
    bm2_d = nc.dram_tensor("b_mem2", [1, HID], F32, kind="ExternalInput").ap()
    out_d = nc.dram_tensor("out", [4 * LD, HID], F32, kind="ExternalOutput").ap()

    with tile.TileContext(nc) as tc:
        with (
            tc.tile_pool(name="const", bufs=1) as cpool,
            tc.tile_pool(name="bc", bufs=1) as bcpool,
            tc.tile_pool(name="wT", bufs=1) as wtpool,
            tc.tile_pool(name="wnat", bufs=1) as wnatpool,
            tc.tile_pool(name="mem", bufs=1) as mempool,
            tc.tile_pool(name="rows", bufs=1) as rowpool,
            tc.tile_pool(name="at", bufs=4) as atpool,
            tc.tile_pool(name="intp", bufs=3) as intpool,
            tc.tile_pool(name="inp2", bufs=4) as inp2pool,
            tc.tile_pool(name="prod", bufs=3) as prodpool,
            tc.tile_pool(name="ttr", bufs=2) as ttrpool,
            tc.tile_pool(name="small", bufs=4) as smallpool,
            tc.tile_pool(name="ptr", bufs=2, space="PSUM") as ptrpool,
            tc.tile_pool(name="pout", bufs=4, space="PSUM") as poutpool,
            tc.tile_pool(name="psS", bufs=1, space="PSUM") as pspool,
        ):
            # ---------------- constants / small loads ----------------
            ident = cpool.tile([P, P], F32)
            make_identity(nc, ident)
            ones_row = cpool.tile([1, P], F32)
            nc.vector.memset(ones_row[:], 1.0)
            ones_col = cpool.tile([P, 1], F32)
            nc.vector.memset(ones_col[:], 1.0)

            wi1_row = rowpool.tile([1, HID], F32, tag="wi1r")
            nc.sync.dma_start(wi1_row[:], wi1_d[:])
            wm1_row = rowpool.tile([1, HID], F32, tag="wm1r")
            nc.sync.dma_start(wm1_row[:], wm1_d[:])
            bi2_row = rowpool.tile([1, HID], F32, tag="bi2r")
            nc.sync.dma_start(bi2_row[:], bi2_d[:])
            bm2_row = rowpool.tile([1, HID], F32, tag="bm2r")
            nc.sync.dma_start(bm2_row[:], bm2_d[:])
            mask_col = cpool.tile([P, MT], F32)
            nc.sync.dma_start(mask_col[:], mask_d.rearrange("1 (o p) -> p o", p=P))

            # broadcasts of rows used inside per-partition ops (DMA bcast)
            wi1_bc = bcpool.tile([P, HID], F32, tag="wi1bc")
            nc.sync.dma_start(wi1_bc[:], wi1_d.to_broadcast([P, HID]))
            wm1_bc = bcpool.tile([P, HID], F32, tag="wm1bc")
            nc.sync.dma_start(wm1_bc[:], wm1_d.to_broadcast([P, HID]))
            bi2_bc = bcpool.tile([P, HID], F32, tag="bi2bc")
            nc.sync.dma_start(bi2_bc[:], bi2_d.to_broadcast([P, HID]))

            # ---------------- W_in2^T via PE transpose ----------------
            wnat = wnatpool.tile([P, KT, HID], F32, tag="wnat")
            wi2_r = Wi2_d.rearrange("(o p) d -> p o d", p=P)
            for k in range(KT):
                nc.sync.dma_start(
                    wnat[:, :, k * P:(k + 1) * P], wi2_r[:, :, k * P:(k + 1) * P]
                )
            w2t = wtpool.tile([P, KT, HID], F32R)
            for k in range(KT):
                for ojh in range(2):
                    ps = ptrpool.tile([P, 512], F32, tag="tr")
                    for j in range(4):
                        oj = ojh * 4 + j
                        nc.tensor.transpose(
                            ps[:, j * P:(j + 1) * P],
                            wnat[:, oj, k * P:(k + 1) * P],
                            ident,
                        )
                    nc.scalar.copy(w2t[:, k, ojh * 512:(ojh + 1) * 512], ps[:])

            # ---------------- v path ----------------
            mem_t = mempool.tile([P, MT, HID], F32, tag="memt")
            nc.sync.dma_start(mem_t[:], mem_d.rearrange("(j p) d -> p j d", p=P))

            # s[m] = memory[m]@w_mem1 - 1e30*(1-mask[m]) ; column layout [128, MT]
            s_col = smallpool.tile([P, MT], F32, tag="scol")
            for j in range(MT):
                scr = ttrpool.tile([P, HID], F32, tag="ttr")
                nc.vector.tensor_mul(scr[:], mem_t[:, j, :], wm1_bc[:])
                nc.vector.tensor_reduce(s_col[:, j:j + 1], scr[:], AX.X, OP.add)
            msk = smallpool.tile([P, MT], F32, tag="msk")
            nc.vector.tensor_scalar(msk[:], mask_col[:], -1.0, 1e30, OP.add, OP.mult)
            nc.vector.tensor_add(s_col[:], s_col[:], msk[:])
            # e_s = exp(s) (no max-shift needed: |s| <= ~5 for live entries)
            e_s = smallpool.tile([P, MT], F32, tag="es")
            nc.scalar.activation(e_s[:], s_col[:], EXP)

            # P_un[o] = sum_m e_s[m] * memory[m, o]  (PE, fp32)
            pun_ps = [ptrpool.tile([1, 512], F32, tag="tr", name=f"pun{h}") for h in range(2)]
            for j in range(MT):
                for h in range(2):
                    nc.tensor.matmul(
                        pun_ps[h][:], e_s[:, j:j + 1],
                        mem_t[:, j, h * 512:(h + 1) * 512],
                        start=(j == 0), stop=(j == MT - 1),
                        skip_group_check=True,
                    )
            pun_row = rowpool.tile([1, HID], F32, tag="punr")
            for h in range(2):
                nc.scalar.copy(pun_row[:, h * 512:(h + 1) * 512], pun_ps[h][:])

            # Z_s = sum_m e_s[m]
            zs_ps = poutpool.tile([1, MT], F32, tag="out")
            nc.tensor.matmul(zs_ps[:], ones_col[:], e_s[:], start=True, stop=True)
            zs_row = smallpool.tile([1, 1], F32, tag="zs")
            nc.vector.tensor_reduce(zs_row[:], zs_ps[:], AX.X, OP.add)
            rzs = smallpool.tile([1, 1], F32, tag="rzs")
            nc.vector.reciprocal(rzs[:], zs_row[:])

            # vT[o] = sum_d W_mem2[o, d] * P_un[d]  (DVE dot vs natural W_mem2)
            pun_bc = _rank1_bcast(nc, bcpool, poutpool, ones_row, pun_row, "punbc")
            wm2nat = wnatpool.tile([P, KT, HID], F32, tag="wnat")
            nc.sync.dma_start(wm2nat[:], Wm2_d.rearrange("(o p) d -> p o d", p=P))
            vt_col = smallpool.tile([P, KT], F32, tag="vt")
            for j2 in range(KT):
                scr = ttrpool.tile([P, HID], F32, tag="ttr")
                nc.vector.tensor_mul(scr[:], wm2nat[:, j2, :], pun_bc[:])
                nc.vector.tensor_reduce(vt_col[:, j2:j2 + 1], scr[:], AX.X, OP.add)
            # v = vT/Z_s + b_mem2 as a [1, HID] row
            v_row_u = rowpool.tile([1, HID], F32, tag="vrowu")
            for j2 in range(KT):
                nc.sync.dma_start(
                    v_row_u[:, j2 * P:(j2 + 1) * P], vt_col[:, j2:j2 + 1]
                )
            v_row = rowpool.tile([1, HID], F32, tag="vrow")
            nc.vector.tensor_scalar(v_row[:], v_row_u[:], rzs[:], None, OP.mult)
            nc.vector.tensor_add(v_row[:], v_row[:], bm2_row[:])
            v_bc = _rank1_bcast(nc, bcpool, poutpool, ones_row, v_row, "vbc")

            # ---------------- main loop (software-pipelined) ----------------
            # Stage skews decouple the PE instruction stream from ACT/DVE
            # producers: matmuls for tile i are emitted one iteration after
            # its transposes (so PE runs tile i+1 transposes while ACT
            # copies int_t_i), and S-accumulation matmuls are emitted three
            # iterations late (so they never wait on the DVE bias-add).
            e_tile = cpool.tile([P, LT], F32R)
            e_f32 = cpool.tile([P, LT], F32)
            s_ps = [pspool.tile([1, 512], F32, tag=f"s{h}", name=f"s{h}") for h in range(2)]
            int_ts = {}
            inp2_sbs = {}

            def emit_load_transpose(i):
                at = atpool.tile([P, HID], F32, tag="at", name=f"at{i}")
                nc.sync.dma_start(at[:], inp_d[i * P:(i + 1) * P, :])

                # idot -> e (exp without shift; |idot| <= ~4)
                scr = ttrpool.tile([P, HID], F32, tag="ttr", name=f"scr{i}")
                idc = smallpool.tile([P, 1], F32, tag="idc", name=f"idc{i}")
                nc.vector.tensor_mul(scr[:], at[:], wi1_bc[:])
                nc.vector.tensor_reduce(idc[:], scr[:], AX.X, OP.add)
                nc.scalar.activation(e_f32[:, i:i + 1], idc[:], EXP)
                nc.scalar.copy(e_tile[:, i:i + 1], e_f32[:, i:i + 1])

                # input tile transpose -> [128(d), k, 128(l)] rounded to f32r
                int_t = intpool.tile([P, KT, P], F32R, tag="int", name=f"int{i}")
                for kh in range(2):
                    ps = ptrpool.tile([P, 512], F32, tag="tr", name=f"trp{i}_{kh}")
                    for j in range(4):
                        k = kh * 4 + j
                        nc.tensor.transpose(
                            ps[:, j * P:(j + 1) * P],
                            at[:, k * P:(k + 1) * P],
                            ident,
                        )
                    nc.scalar.copy(
                        int_t[:, kh * 4:(kh + 1) * 4, :].rearrange("p a b -> p (a b)"),
                        ps[:],
                    )
                int_ts[i] = int_t

            def emit_mm(i):
                int_t = int_ts.pop(i)
                inp2_sb = inp2pool.tile([P, HID], F32R, tag="inp2", name=f"i2_{i}")
                for h in range(2):
                    pso = poutpool.tile([P, 512], F32, tag="out", name=f"pso{i}_{h}")
                    for k in range(KT):
                        nc.tensor.matmul(
                            pso[:], int_t[:, k, :],
                            w2t[:, k, h * 512:(h + 1) * 512],
                            start=(k == 0), stop=(k == KT - 1),
                        )
                    nc.vector.tensor_add(
                        inp2_sb[:, h * 512:(h + 1) * 512], pso[:],
                        bi2_bc[:, h * 512:(h + 1) * 512],
                    )
                prod_sb = prodpool.tile([P, HID], F32, tag="prod", name=f"pr{i}")
                nc.vector.tensor_mul(prod_sb[:], inp2_sb.bitcast(F32)[:], v_bc[:])
                nc.sync.dma_start(
                    out_d[i * P:(i + 1) * P, :], inp2_sb.bitcast(F32)[:]
                )
                nc.sync.dma_start(
                    out_d[2 * LD + i * P:2 * LD + (i + 1) * P, :], prod_sb[:]
                )
                nc.sync.dma_start(out_d[LD + i * P:LD + (i + 1) * P, :], v_bc[:])
                inp2_sbs[i] = inp2_sb

            def emit_s(i):
                inp2_sb = inp2_sbs.pop(i)
                for h in range(2):
                    # S += e_i^T inp2_i  (accumulates over the whole loop)
                    nc.tensor.matmul(
                        s_ps[h][:], e_tile[:, i:i + 1],
                        inp2_sb[:, h * 512:(h + 1) * 512],
                        start=(i == 0), stop=(i == LT - 1),
                        skip_group_check=True,
                    )

            MM_SKEW, S_SKEW = 1, 3
            for i in range(LT + S_SKEW):
                if i < LT:
                    emit_load_transpose(i)
                if MM_SKEW <= i < LT + MM_SKEW:
                    emit_mm(i - MM_SKEW)
                if i >= S_SKEW:
                    emit_s(i - S_SKEW)

            # ---------------- tail: out2 and u rows ----------------
            z_ps = poutpool.tile([1, LT], F32, tag="out")
            nc.tensor.matmul(z_ps[:], ones_col[:], e_f32[:], start=True, stop=True)
            z_row = smallpool.tile([1, LT], F32, tag="zrow")
            nc.scalar.copy(z_row[:], z_ps[:])
            z_sb = smallpool.tile([1, 1], F32, tag="z")
            nc.vector.tensor_reduce(z_sb[:], z_row[:], AX.X, OP.add)
            rz = smallpool.tile([1, 1], F32, tag="rz")
            nc.vector.reciprocal(rz[:], z_sb[:])
            s_row = rowpool.tile([1, HID], F32, tag="srow")
            for h in range(2):
                nc.scalar.copy(s_row[:, h * 512:(h + 1) * 512], s_ps[h][:])
            u_row = rowpool.tile([1, HID], F32, tag="urow")
            nc.vector.tensor_scalar(u_row[:], s_row[:], rz[:], None, OP.mult)
            nc.vector.tensor_mul(u_row[:], u_row[:], v_row[:])
            u_bc = _rank1_bcast(nc, bcpool, poutpool, ones_row, u_row, "ubc")
            for i in range(LT):
                nc.sync.dma_start(
                    out_d[3 * LD + i * P:3 * LD + (i + 1) * P, :], u_bc[:]
                )

    nc.finalize()
    return nc


def _get_nc():
    global _NC_CACHE
    if _NC_CACHE is None:
        _NC_CACHE = _build_nc()
    return _NC_CACHE


def kernel(**inputs) -> np.ndarray:
    nc = _get_nc()
    inp = np.asarray(inputs["input"], dtype=np.float32)
    mem = np.asarray(inputs["memory"], dtype=np.float32)
    mask = np.asarray(inputs["mask"], dtype=np.float32)
    w_in1 = np.ascontiguousarray(np.asarray(inputs["w_in1"], np.float32).reshape(1, HID))
    w_mem1 = np.ascontiguousarray(np.asarray(inputs["w_mem1"], np.float32).reshape(1, HID))
    W_in2 = np.ascontiguousarray(np.asarray(inputs["W_in2"], np.float32))
    b_in2 = np.ascontiguousarray(np.asarray(inputs["b_in2"], np.float32).reshape(1, HID))
    W_mem2 = np.ascontiguousarray(np.asarray(inputs["W_mem2"], np.float32))
    b_mem2 = np.ascontiguousarray(np.asarray(inputs["b_mem2"], np.float32).reshape(1, HID))

    in_maps = []
    for b in range(N_CORES):
        in_maps.append({
            "input": np.ascontiguousarray(inp[b]),
            "memory": np.ascontiguousarray(mem[b]),
            "mask": np.ascontiguousarray(mask[b].reshape(1, LM)),
            "w_in1": w_in1,
            "w_mem1": w_mem1,
            "W_in2": W_in2,
            "b_in2": b_in2,
            "W_mem2": W_mem2,
            "b_mem2": b_mem2,
        })

    res = run_bass_kernel_spmd(nc, in_maps, core_ids=list(range(N_CORES)))
    return np.stack([res.results[c]["out"] for c in range(N_CORES)], axis=0)


# revision 11
# speedup vs baseline: 1.1769x; 1.1306x over previous
"""Trainium2 Bass kernel for nn_BiAttention (sparse_attention).

Math: the reference's attention matrix is rank-1 plus a mask bias:
    att[b,l,m] = input_dot[b,l] + s[b,m],  s[m] = memory[m]@w_mem1 - 1e30*(1-mask[m])
Row softmax over m is invariant to the per-row constant input_dot[b,l], so
    weight_one[b,l,:] = softmax_m(s)            (same for every l)
    output_one[b,l,:] = v_b := softmax_m(s) @ (memory @ W_mem2.T + b_mem2)
Likewise max_m att[b,l,m] = input_dot[b,l] + const, so
    weight_two[b,0,:] = softmax_l(input_dot)
    output_two[b,0,:] = softmax_l(input_dot) @ inp2
The output [N, 4*Ld, d] row blocks are:
    [0:2048]    inp2 = input @ W_in2.T + b_in2
    [2048:4096] v_b broadcast
    [4096:6144] inp2 * v_b
    [6144:8192] (output_two * v_b) broadcast

Sharding: pure data parallel, one batch element per NeuronCore (8 cores).

Schedule notes: engine sequencers and HWDGE rings are strict FIFO, so
emission order is scheduling. Reads go on the ACT DMA ring, writes on
the SP ring. The big matmul runs in float32r (full PE rate, ~1e-4 rel).
The v path (W_mem2) is computed via PE transposes + f32r matvecs and is
spliced between the first main-loop tiles; prod (inp2*v) is deferred a
few tiles so the DVE FIFO never blocks on v_bc.
"""

import numpy as np

import concourse.bass as bass
import concourse.tile as tile
from concourse import bacc, mybir
from concourse.bass_utils import run_bass_kernel_spmd
from concourse.masks import make_identity

F32 = mybir.dt.float32
F32R = mybir.dt.float32r
AX = mybir.AxisListType
OP = mybir.AluOpType
EXP = mybir.ActivationFunctionType.Exp

P = 128
BSZ, LD, LM, HID = 8, 2048, 512, 1024
KT = HID // P          # 8 hidden-dim chunks
LT = LD // P           # 16 l tiles
MT = LM // P           # 4 memory tiles
N_CORES = 8

MM_SKEW = 1            # matmuls for tile i emitted 1 iter after transposes
S_SKEW = 3             # S-accumulation matmuls trail by 3 iters
PROD_SKEW = 5          # prod (needs v_bc) trails by 7 iters

_NC_CACHE = None


def _build_nc():
    nc = bacc.Bacc("TRN2", target_bir_lowering=False, num_devices=N_CORES)

    inp_d = nc.dram_tensor("input", [LD, HID], F32, kind="ExternalInput").ap()
    mem_d = nc.dram_tensor("memory", [LM, HID], F32, kind="ExternalInput").ap()
    mask_d = nc.dram_tensor("mask", [1, LM], F32, kind="ExternalInput").ap()
    wi1_d = nc.dram_tensor("w_in1", [1, HID], F32, kind="ExternalInput").ap()
    wm1_d = nc.dram_tensor("w_mem1", [1, HID], F32, kind="ExternalInput").ap()
    Wi2_d = nc.dram_tensor("W_in2", [HID, HID], F32, kind="ExternalInput").ap()
    bi2_d = nc.dram_tensor("b_in2", [1, HID], F32, kind="ExternalInput").ap()
    Wm2_d = nc.dram_tensor("W_mem2", [HID, HID], F32, kind="ExternalInput").ap()
    bm2_d = nc.dram_tensor("b_mem2", [1, HID], F32, kind="ExternalInput").ap()
    out_d = nc.dram_tensor("out", [4 * LD, HID], F32, kind="ExternalOutput").ap()

    with tile.TileContext(nc) as tc:
        with (
            tc.tile_pool(name="const", bufs=1) as cpool,
            tc.tile_pool(name="bc", bufs=1) as bcpool,
            tc.tile_pool(name="wT", bufs=1) as wtpool,
            tc.tile_pool(name="wnat", bufs=1) as wnatpool,
            tc.tile_pool(name="mem", bufs=1) as mempool,
            tc.tile_pool(name="rows", bufs=1) as rowpool,
            tc.tile_pool(name="at", bufs=4) as atpool,
            tc.tile_pool(name="intp", bufs=3) as intpool,
            tc.tile_pool(name="inp2", bufs=6) as inp2pool,
            tc.tile_pool(name="prod", bufs=2) as prodpool,
            tc.tile_pool(name="ttr", bufs=1) as ttrpool,
            tc.tile_pool(name="small", bufs=4) as smallpool,
            tc.tile_pool(name="ptr", bufs=2, space="PSUM") as ptrpool,
            tc.tile_pool(name="pout", bufs=4, space="PSUM") as poutpool,
            tc.tile_pool(name="psS", bufs=1, space="PSUM") as pspool,
        ):
            # ---------------- constants & small loads (ACT ring) ----------
            ident = cpool.tile([P, P], F32)
            make_identity(nc, ident)
            ones_row = cpool.tile([1, P], F32)
            nc.vector.memset(ones_row[:], 1.0)
            ones_col = cpool.tile([P, 1], F32)
            nc.vector.memset(ones_col[:], 1.0)

            bm2_row = rowpool.tile([1, HID], F32, tag="bm2r")
            nc.scalar.dma_start(bm2_row[:], bm2_d[:])
            mask_col = cpool.tile([P, MT], F32)
            nc.scalar.dma_start(mask_col[:], mask_d.rearrange("1 (o p) -> p o", p=P))
            wm1_bc = bcpool.tile([P, HID], F32, tag="wm1bc")
            nc.scalar.dma_start(wm1_bc[:], wm1_d.to_broadcast([P, HID]))

            # ---------------- helpers ----------------
            at_tiles = {}
            int_ts = {}
            inp2_sbs = {}
            e_tile = cpool.tile([P, LT], F32R)
            e_f32 = cpool.tile([P, LT], F32)
            s_ps = [pspool.tile([1, 512], F32, tag=f"s{h}", name=f"s{h}")
                    for h in range(2)]

            def emit_at(i):
                at = atpool.tile([P, HID], F32, tag="at", name=f"at{i}")
                nc.scalar.dma_start(at[:], inp_d[i * P:(i + 1) * P, :])
                at_tiles[i] = at

            def emit_tr_idot(i):
                at = at_tiles.pop(i)
                int_t = intpool.tile([P, KT, P], F32R, tag="int", name=f"int{i}")
                for kh in range(2):
                    ps = ptrpool.tile([P, 512], F32, tag="tr", name=f"trp{i}_{kh}")
                    for j in range(4):
                        k = kh * 4 + j
                        nc.tensor.transpose(
                            ps[:, j * P:(j + 1) * P],
                            at[:, k * P:(k + 1) * P],
                            ident,
                        )
                    nc.scalar.copy(
                        int_t[:, kh * 4:(kh + 1) * 4, :].rearrange("p a b -> p (a b)"),
                        ps[:],
                    )
                int_ts[i] = int_t
                # idot in place: at is dead after the transposes above
                idc = smallpool.tile([P, 1], F32, tag="idc", name=f"idc{i}")
                nc.vector.tensor_mul(at[:], at[:], wi1_bc[:])
                nc.vector.tensor_reduce(idc[:], at[:], AX.X, OP.add)
                nc.scalar.activation(e_f32[:, i:i + 1], idc[:], EXP)
                nc.scalar.copy(e_tile[:, i:i + 1], e_f32[:, i:i + 1])

            def emit_mm(i):
                int_t = int_ts.pop(i)
                inp2_sb = inp2pool.tile([P, HID], F32R, tag="inp2", name=f"i2_{i}")
                for h in range(2):
                    pso = poutpool.tile([P, 512], F32, tag="out", name=f"pso{i}_{h}")
                    for k in range(KT):
                        nc.tensor.matmul(
                            pso[:], int_t[:, k, :],
                            w2t[:, k, h * 512:(h + 1) * 512],
                            start=(k == 0), stop=(k == KT - 1),
                        )
                    nc.vector.tensor_add(
                        inp2_sb[:, h * 512:(h + 1) * 512], pso[:],
                        bi2_bc[:, h * 512:(h + 1) * 512],
                    )
                nc.sync.dma_start(
                    out_d[i * P:(i + 1) * P, :], inp2_sb.bitcast(F32)[:]
                )
                inp2_sbs[i] = inp2_sb

            def emit_s(i):
                for h in range(2):
                    nc.tensor.matmul(
                        s_ps[h][:], e_tile[:, i:i + 1],
                        inp2_sbs[i][:, h * 512:(h + 1) * 512],
                        start=(i == 0), stop=(i == LT - 1),
                        skip_group_check=True,
                    )

            def emit_prod(i):
                inp2_sb = inp2_sbs.pop(i)
                prod_sb = prodpool.tile([P, HID], F32, tag="prod", name=f"pr{i}")
                nc.vector.tensor_mul(prod_sb[:], inp2_sb.bitcast(F32)[:], v_bc[:])
                nc.sync.dma_start(
                    out_d[2 * LD + i * P:2 * LD + (i + 1) * P, :], prod_sb[:]
                )

            def transpose_1024(dst, src_nat, tag):
                """dst[:, k, oj*128: ] = transpose of src_nat[:, oj, k*128: ]."""
                for k in range(KT):
                    for ojh in range(2):
                        ps = ptrpool.tile([P, 512], F32, tag="tr",
                                          name=f"{tag}{k}_{ojh}")
                        for j in range(4):
                            oj = ojh * 4 + j
                            nc.tensor.transpose(
                                ps[:, j * P:(j + 1) * P],
                                src_nat[:, oj, k * P:(k + 1) * P],
                                ident,
                            )
                        nc.scalar.copy(
                            dst[:, k, ojh * 512:(ojh + 1) * 512], ps[:]
                        )

            def rank1_bcast(row_ap, name):
                bc = bcpool.tile([P, HID], F32, tag=name, name=name)
                for h in range(2):
                    ps = poutpool.tile([P, 512], F32, tag="out", name=f"{name}{h}")
                    nc.tensor.matmul(
                        ps[:], ones_row[:], row_ap[:, h * 512:(h + 1) * 512],
                        start=True, stop=True,
                    )
                    nc.scalar.copy(bc[:, h * 512:(h + 1) * 512], ps[:])
                return bc

            # ---------------- W_in2 chunks + first input tiles ------------
            wnat = wnatpool.tile([P, KT, HID], F32, tag="wnat", name="wnat")
            wi2_r = Wi2_d.rearrange("(o p) d -> p o d", p=P)

            def w_chunk(k):
                nc.scalar.dma_start(
                    wnat[:, :, k * P:(k + 1) * P], wi2_r[:, :, k * P:(k + 1) * P]
                )

            w_chunk(0); w_chunk(1)
            emit_at(0)
            w_chunk(2); w_chunk(3)
            # memory for the s path (DVE work, overlaps W transposes)
            mem_t = mempool.tile([P, MT, HID], F32, tag="memt")
            nc.scalar.dma_start(mem_t[:], mem_d.rearrange("(j p) d -> p j d", p=P))
            w_chunk(4); w_chunk(5)
            emit_at(1)
            w_chunk(6); w_chunk(7)
            wi1_bc = bcpool.tile([P, HID], F32, tag="wi1bc")
            nc.scalar.dma_start(wi1_bc[:], wi1_d.to_broadcast([P, HID]))
            emit_at(2)
            bi2_bc = bcpool.tile([P, HID], F32, tag="bi2bc")
            nc.scalar.dma_start(bi2_bc[:], bi2_d.to_broadcast([P, HID]))

            # W_in2^T on PE (fills the head while DMAs stream)
            w2t = wtpool.tile([P, KT, HID], F32R, tag="w2t")
            transpose_1024(w2t, wnat, "wt")

            # s path on DVE (front of the DVE FIFO; memory arrives first)
            s_col = smallpool.tile([P, MT], F32, tag="scol")
            for j in range(MT):
                scr = ttrpool.tile([P, HID], F32, tag="ttr", name=f"sscr{j}")
                nc.vector.tensor_mul(scr[:], mem_t[:, j, :], wm1_bc[:])
                nc.vector.tensor_reduce(s_col[:, j:j + 1], scr[:], AX.X, OP.add)
            msk = smallpool.tile([P, MT], F32, tag="msk")
            nc.vector.tensor_scalar(msk[:], mask_col[:], -1.0, 1e30, OP.add, OP.mult)
            nc.vector.tensor_add(s_col[:], s_col[:], msk[:])
            e_s = smallpool.tile([P, MT], F32, tag="es")
            nc.scalar.activation(e_s[:], s_col[:], EXP)

            emit_tr_idot(0)

            # P_un[o] = sum_m e_s[m] * memory[m, o]  (PE, fp32) and Z_s
            pun_row = rowpool.tile([1, HID], F32, tag="punr")
            for h in range(2):
                pun_ps = ptrpool.tile([1, 512], F32, tag="tr", name=f"pun{h}")
                for j in range(MT):
                    nc.tensor.matmul(
                        pun_ps[:], e_s[:, j:j + 1],
                        mem_t[:, j, h * 512:(h + 1) * 512],
                        start=(j == 0), stop=(j == MT - 1),
                        skip_group_check=True,
                    )
                nc.scalar.copy(pun_row[:, h * 512:(h + 1) * 512], pun_ps[:])
            zs_ps = poutpool.tile([1, MT], F32, tag="out")
            nc.tensor.matmul(zs_ps[:], ones_col[:], e_s[:], start=True, stop=True)
            zs_row = smallpool.tile([1, 1], F32, tag="zs")
            nc.vector.tensor_reduce(zs_row[:], zs_ps[:], AX.X, OP.add)
            rzs = smallpool.tile([1, 1], F32, tag="rzs")
            nc.vector.reciprocal(rzs[:], zs_row[:])

            emit_tr_idot(1)
            emit_mm(0)

            # ---------------- W_mem2^T + v path (PE matvecs) --------------
            wm2nat = wnatpool.tile([P, KT, HID], F32, tag="wnat", name="wm2nat")
            wm2_r = Wm2_d.rearrange("(o p) d -> p o d", p=P)

            def wm2_chunk(k):
                nc.scalar.dma_start(
                    wm2nat[:, :, k * P:(k + 1) * P], wm2_r[:, :, k * P:(k + 1) * P]
                )

            wm2_chunk(0); wm2_chunk(1); wm2_chunk(2)
            emit_at(3)
            wm2_chunk(3); wm2_chunk(4)
            emit_at(4)
            wm2_chunk(5); wm2_chunk(6)
            emit_at(5)
            wm2_chunk(7)
            emit_tr_idot(2)
            emit_mm(1)

            # p as f32r column chunks [128, KT]
            p_col_f = smallpool.tile([P, KT], F32, tag="pcolf")
            for j in range(KT):
                nc.sync.dma_start(
                    p_col_f[:, j:j + 1], pun_row[:, j * P:(j + 1) * P]
                )
            p_col = smallpool.tile([P, KT], F32R, tag="pcol")
            nc.vector.tensor_copy(p_col[:], p_col_f[:])

            emit_at(4)
            emit_tr_idot(3)
            emit_mm(2)
            emit_s(0)

            # v_unb[o] = sum_d p[d] * W_mem2^T[d, o]  (f32r matvecs), with
            # W_mem2^T built half-at-a-time to halve its SBUF footprint
            v_row = rowpool.tile([1, HID], F32, tag="vrow")
            for h in range(2):
                wm2t = wtpool.tile([P, KT, 512], F32R, tag="wm2t",
                                   name=f"wm2t{h}")
                for k in range(KT):
                    ps = ptrpool.tile([P, 512], F32, tag="tr",
                                      name=f"wmt{k}_{h}")
                    for j in range(4):
                        oj = h * 4 + j
                        nc.tensor.transpose(
                            ps[:, j * P:(j + 1) * P],
                            wm2nat[:, oj, k * P:(k + 1) * P],
                            ident,
                        )
                    nc.scalar.copy(wm2t[:, k, :], ps[:])
                v_ps = ptrpool.tile([1, 512], F32, tag="tr", name=f"vps{h}")
                for k in range(KT):
                    nc.tensor.matmul(
                        v_ps[:], p_col[:, k:k + 1],
                        wm2t[:, k, :],
                        start=(k == 0), stop=(k == KT - 1),
                        skip_group_check=True,
                    )
                nc.scalar.copy(v_row[:, h * 512:(h + 1) * 512], v_ps[:])
            nc.vector.tensor_scalar(v_row[:], v_row[:], rzs[:], None, OP.mult)
            nc.vector.tensor_add(v_row[:], v_row[:], bm2_row[:])
            v_bc = rank1_bcast(v_row, "vbc")

            # ---------------- steady-state loop ----------------
            # iteration i: at(i+2), tr/idot(i), mm(i-1), s(i-3), prod(i-6),
            # plus v-row writes spread out
            vw = [0]

            def emit_vwrite():
                i = vw[0]
                if i < LT:
                    nc.sync.dma_start(
                        out_d[LD + i * P:LD + (i + 1) * P, :], v_bc[:]
                    )
                    vw[0] += 1

            for i in range(4, LT + PROD_SKEW):
                if i + 2 < LT:
                    emit_at(i + 2)
                if i < LT:
                    emit_tr_idot(i)
                if i - MM_SKEW < LT:
                    emit_mm(i - MM_SKEW)
                if 0 <= i - S_SKEW < LT:
                    emit_s(i - S_SKEW)
                if i >= PROD_SKEW:
                    emit_prod(i - PROD_SKEW)
                emit_vwrite()
                emit_vwrite()

            while vw[0] < LT:
                emit_vwrite()

            # ---------------- tail: out2 and u rows ----------------
            z_ps = poutpool.tile([1, LT], F32, tag="out")
            nc.tensor.matmul(z_ps[:], ones_col[:], e_f32[:], start=True, stop=True)
            z_row = smallpool.tile([1, LT], F32, tag="zrow")
            nc.scalar.copy(z_row[:], z_ps[:])
            z_sb = smallpool.tile([1, 1], F32, tag="z")
            nc.vector.tensor_reduce(z_sb[:], z_row[:], AX.X, OP.add)
            rz = smallpool.tile([1, 1], F32, tag="rz")
            nc.vector.reciprocal(rz[:], z_sb[:])
            s_row = rowpool.tile([1, HID], F32, tag="srow")
            for h in range(2):
                nc.scalar.copy(s_row[:, h * 512:(h + 1) * 512], s_ps[h][:])
            u_row = rowpool.tile([1, HID], F32, tag="urow")
            nc.vector.tensor_scalar(u_row[:], s_row[:], rz[:], None, OP.mult)
            nc.vector.tensor_mul(u_row[:], u_row[:], v_row[:])
            u_bc = rank1_bcast(u_row, "ubc")
            for i in range(LT):
                nc.sync.dma_start(
                    out_d[3 * LD + i * P:3 * LD + (i + 1) * P, :], u_bc[:]
                )

    nc.finalize()
    return nc


def _get_nc():
    global _NC_CACHE
    if _NC_CACHE is None:
        _NC_CACHE = _build_nc()
    return _NC_CACHE


def kernel(**inputs) -> np.ndarray:
    nc = _get_nc()
    inp = np.asarray(inputs["input"], dtype=np.float32)
    mem = np.asarray(inputs["memory"], dtype=np.float32)
    mask = np.asarray(inputs["mask"], dtype=np.float32)
    w_in1 = np.ascontiguousarray(np.asarray(inputs["w_in1"], np.float32).reshape(1, HID))
    w_mem1 = np.ascontiguousarray(np.asarray(inputs["w_mem1"], np.float32).reshape(1, HID))
    W_in2 = np.ascontiguousarray(np.asarray(inputs["W_in2"], np.float32))
    b_in2 = np.ascontiguousarray(np.asarray(inputs["b_in2"], np.float32).reshape(1, HID))
    W_mem2 = np.ascontiguousarray(np.asarray(inputs["W_mem2"], np.float32))
    b_mem2 = np.ascontiguousarray(np.asarray(inputs["b_mem2"], np.float32).reshape(1, HID))

    in_maps = []
    for b in range(N_CORES):
        in_maps.append({
            "input": np.ascontiguousarray(inp[b]),
            "memory": np.ascontiguousarray(mem[b]),
            "mask": np.ascontiguousarray(mask[b].reshape(1, LM)),
            "w_in1": w_in1,
            "w_mem1": w_mem1,
            "W_in2": W_in2,
            "b_in2": b_in2,
            "W_mem2": W_mem2,
            "b_mem2": b_mem2,
        })

    res = run_bass_kernel_spmd(nc, in_maps, core_ids=list(range(N_CORES)))
    return np.stack([res.results[c]["out"] for c in range(N_CORES)], axis=0)
